# revision 7
# baseline (speedup 1.0000x reference)
"""ColBERT pairwise-distill KL loss on 8 Trainium2 NeuronCores.

Strategy (doc-axis sharding): core k owns doc batches c in [8k, 8k+8).
Each core holds the full (transposed) query embeddings for student and
teacher in SBUF, streams its doc shard once from HBM, and computes the
full 64-row x 8-col block of both MaxSim score matrices:

    scores[b, c] = sum_n max_s  q[b, n, :] . d[c, s, :]

via float32r (fp22, full-rate) matmuls contracting over D=128 on the
partition axis.  The max over doc tokens runs on the vector engine with
a custom two-source fused max+max-reduce DVE op (2 PSUM elements per
lane-cycle, 2x the builtin tensor_reduce).  The sum over query tokens
is a ones-matrix matmul on the tensor engine.  Host-side work is only
sharding/layout of inputs and the [64,64]->scalar loss epilogue (exact
numpy float32 replica of the jax reference semantics, ~40K flops).
"""

import numpy as np

import concourse.bass as bass
import concourse.tile as tile
from concourse import bacc, mybir
from concourse import dve_ops
from concourse.bass_utils import run_bass_kernel_spmd
from concourse.dve_ops import DveOp
from concourse.dve_spec import C0, Spec, Src0, Src1, lower, maxx
from concourse.dve_uop import DveOpSpec

N_CORES = 8
B, N, S, D = 64, 32, 1024, 128
CPC = B // N_CORES          # doc batches per core
BPG = 4                     # query batches per matmul group (4*32 = 128 rows)
G = B // BPG                # 16 groups
NCOL = G * CPC              # 128 mx columns per side
ALPHA = 0.5
TEMPERATURE = 1.0
F32 = mybir.dt.float32
F32R = mybir.dt.float32r
BF16 = mybir.dt.bfloat16


def _ref_maxx2(in0, in1, c0, c1, c2):
    m = np.maximum(in0.astype(np.float32), in1.astype(np.float32))
    acc = np.maximum(m.reshape(m.shape[0], -1).max(axis=1, keepdims=True), c0)
    return m, acc


def _register_maxx2():
    """out = max(in0, in1); accum_out = max(s0, rowmax(out)).

    Registered through the supported custom-DVE extension point
    (dve_ops.OPS); the uop program ships inside the NEFF.
    """
    name = "MAXX2_REDUCE_ANT"
    if name in dve_ops._SUB_OPCODE_FOR_NAME:
        return next(o for o in dve_ops.OPS if o.name == name)
    spec = Spec(body=maxx(Src0, Src1), accum=maxx, accum_init=C0, reference=_ref_maxx2)
    row = dve_ops._CUSTOM_DVE_ROW_BASE + len(dve_ops.OPS)
    op = DveOp(name, spec, subdim=False, uops_sha={})
    dve_ops.OPS.append(op)
    dve_ops.CUSTOM_DVE_SPECS[name] = spec
    dve_ops._SUB_OPCODE_FOR_NAME[name] = row
    for ver in ("v3",):
        dve_ops._COMPILE_CACHE[(name, ver)] = DveOpSpec(
            name=name, opcode=row, uops=lower(spec, ver=ver), rd1_en=True
        )
    return op


def _emit(tc, nc, maxx2, qT_d, tqT_d, dT_d, tdT_d, ones4_d, out_d, ctx):
    sides = ((qT_d, dT_d), (tqT_d, tdT_d))

    resident = ctx.enter_context(tc.tile_pool(name="resident", bufs=1))
    scr = ctx.enter_context(tc.tile_pool(name="scr", bufs=2))

    ones_sb = resident.tile([D, BPG], F32, tag="ones", name="ones_sb")
    nc.sync.dma_start(ones_sb[:], ones4_d.ap())

    # Per-side resident operands + the per-(group, doc-batch) max matrix.
    qt_sb, dt_sb, mx_sb = [], [], []
    for si, (q_d, d_d) in enumerate(sides):
        qt = resident.tile([D, B * N], BF16, tag=f"qt{si}", name=f"qt{si}")
        nc.sync.dma_start(qt[:], q_d.ap())
        qt_sb.append(qt)
        dts = []
        for c in range(CPC):
            dt = resident.tile([D, S], BF16, tag=f"dt{si}_{c}", name=f"dt{si}_{c}")
            nc.sync.dma_start(dt[:], d_d.ap()[:, c, :])
            dts.append(dt)
        dt_sb.append(dts)
        mx_sb.append(resident.tile([D, NCOL], F32, tag=f"mx{si}", name=f"mx{si}"))

    half = S // 2
    main_psum = tc.tile_pool(name="psum", bufs=2, space="PSUM")
    psum = main_psum.__enter__()
    for si in range(2):
        for g in range(G):
            lhsT = qt_sb[si][:, g * 128 : (g + 1) * 128]
            for c0 in range(0, CPC, 2):
                # One 4-bank PSUM tile holds two doc batches (c0, c0+1).
                ps = psum.tile([D, 2 * S], F32)
                for j in range(2):
                    dt = dt_sb[si][c0 + j]
                    nc.tensor.matmul(
                        ps[:, j * S : j * S + half], lhsT, dt[:, 0:half],
                        start=True, stop=True,
                    )
                    nc.tensor.matmul(
                        ps[:, j * S + half : (j + 1) * S], lhsT, dt[:, half:S],
                        start=True, stop=True,
                    )
                # ScalarE stages both second halves into SBUF in one batched
                # copy (DVE can read at most one PSUM operand per instruction).
                sb2 = scr.tile([D, 2, half], F32, name="sb2")
                ps3 = ps[:].rearrange("p (c s) -> p c s", c=2)
                nc.scalar.copy(sb2[:], ps3[:, :, half:S])
                for j in range(2):
                    col = g * CPC + c0 + j
                    scratch = scr.tile([D, half], F32)
                    nc.vector._custom_dve(
                        maxx2,
                        out=scratch[:],
                        in0=ps[:, j * S : j * S + half],
                        in1=sb2[:, j, :],
                        s0=-3.0e38,
                        accum_out=mx_sb[si][:, col : col + 1],
                    )

    main_psum.__exit__(None, None, None)

    # Sum over the 32 query tokens inside each 32-partition block:
    # scores_part[p, col] for the 4 batches of each group.
    out_sb = resident.tile([BPG, 2 * NCOL], F32, tag="out", name="out_sb")
    with tc.tile_pool(name="psum_fin", bufs=1, space="PSUM") as psum_fin:
        psf = psum_fin.tile([BPG, 2 * NCOL], F32)
        for si in range(2):
            nc.tensor.matmul(
                psf[:, si * NCOL : (si + 1) * NCOL], ones_sb[:], mx_sb[si][:],
                start=True, stop=True,
            )
        nc.scalar.copy(out_sb[:], psf[:])
    nc.sync.dma_start(out_d.ap(), out_sb[:])


_PROGRAM = None


def _build_program():
    global _PROGRAM
    if _PROGRAM is not None:
        return _PROGRAM
    maxx2 = _register_maxx2()
    nc = bacc.Bacc("TRN2", target_bir_lowering=False, debug=False, num_devices=N_CORES)
    qT_d = nc.dram_tensor("qT", [D, B * N], BF16, kind="ExternalInput")
    tqT_d = nc.dram_tensor("tqT", [D, B * N], BF16, kind="ExternalInput")
    dT_d = nc.dram_tensor("dT", [D, CPC, S], BF16, kind="ExternalInput")
    tdT_d = nc.dram_tensor("tdT", [D, CPC, S], BF16, kind="ExternalInput")
    ones4_d = nc.dram_tensor("ones4", [D, BPG], F32, kind="ExternalInput")
    out_d = nc.dram_tensor("out", [BPG, 2 * NCOL], F32, kind="ExternalOutput")

    from contextlib import ExitStack

    with tile.TileContext(nc) as tc:
        with ExitStack() as ctx:
            _emit(tc, nc, maxx2, qT_d, tqT_d, dT_d, tdT_d, ones4_d, out_d, ctx)
    nc.compile()
    _PROGRAM = nc
    return nc


def _make_in_maps(query_embeddings, doc_embeddings, teacher_query_outputs,
                  teacher_doc_outputs):
    import ml_dtypes
    bf16 = ml_dtypes.bfloat16
    # [B, N, D] -> [D, B*N]; column index = 32*b + n = 128*g + 32*(b%4) + n.
    qT = np.ascontiguousarray(
        np.asarray(query_embeddings, dtype=np.float32).transpose(2, 0, 1).reshape(D, B * N)
    ).astype(bf16)
    tqT = np.ascontiguousarray(
        np.asarray(teacher_query_outputs, dtype=np.float32).transpose(2, 0, 1).reshape(D, B * N)
    ).astype(bf16)
    d_full = np.asarray(doc_embeddings, dtype=np.float32)
    td_full = np.asarray(teacher_doc_outputs, dtype=np.float32)
    ones4 = np.zeros((D, BPG), dtype=np.float32)
    for m in range(BPG):
        ones4[32 * m : 32 * (m + 1), m] = 1.0

    in_maps = []
    for k in range(N_CORES):
        sl = slice(k * CPC, (k + 1) * CPC)
        in_maps.append({
            "qT": qT,
            "tqT": tqT,
            "dT": np.ascontiguousarray(d_full[sl].transpose(2, 0, 1)).astype(bf16),
            "tdT": np.ascontiguousarray(td_full[sl].transpose(2, 0, 1)).astype(bf16),
            "ones4": ones4,
        })
    return in_maps


def _assemble_scores(results):
    """Per-core out [4, 256] -> full scores [64, 64] (student, teacher)."""
    s = np.empty((B, B), dtype=np.float32)
    ts = np.empty((B, B), dtype=np.float32)
    for k, r in enumerate(results):
        o = np.asarray(r["out"], dtype=np.float32)
        for si, dst in enumerate((s, ts)):
            blk = o[:, si * NCOL : (si + 1) * NCOL].reshape(BPG, G, CPC)
            # blk[p, g, cl] = scores[4g + p, 8k + cl]
            dst[:, k * CPC : (k + 1) * CPC] = (
                blk.transpose(1, 0, 2).reshape(B, CPC)
            )
    return s, ts


def _loss_from_scores(s, ts):
    """Exact numpy float32 replica of the jax reference epilogue."""
    pos = np.diagonal(s).copy()
    masked = s.copy()
    np.fill_diagonal(masked, -np.inf)
    neg = masked.max(axis=1)
    contrastive = np.logaddexp(np.float32(0.0), neg - pos).astype(np.float32).mean(
        dtype=np.float32
    )

    m = s.max(axis=1, keepdims=True)
    sh = s - m
    lse = np.log(np.exp(sh).sum(axis=1, keepdims=True, dtype=np.float32))
    slp = sh - lse

    mt = ts.max(axis=1, keepdims=True)
    e = np.exp(ts - mt)
    tp = (e / e.sum(axis=1, keepdims=True, dtype=np.float32)).astype(np.float32)
    with np.errstate(divide="ignore", invalid="ignore"):
        logtp = np.log(tp)
        kl = np.sum(tp * (logtp - slp), dtype=np.float32) / np.float32(B)
    kl = kl * np.float32(TEMPERATURE) ** 2
    return np.float32(contrastive + np.float32(ALPHA) * kl)


def run_scores(query_embeddings, doc_embeddings, teacher_query_outputs,
               teacher_doc_outputs, **spmd_kwargs):
    """Run the device program; returns (scores, teacher_scores, raw results)."""
    nc = _build_program()
    in_maps = _make_in_maps(
        query_embeddings, doc_embeddings, teacher_query_outputs, teacher_doc_outputs
    )
    res = run_bass_kernel_spmd(nc, in_maps, core_ids=list(range(N_CORES)), **spmd_kwargs)
    s, ts = _assemble_scores(res.results)
    return s, ts, res


def kernel(query_embeddings, doc_embeddings, teacher_query_outputs,
           teacher_doc_outputs):
    s, ts, _ = run_scores(
        query_embeddings, doc_embeddings, teacher_query_outputs, teacher_doc_outputs
    )
    return np.array(_loss_from_scores(s, ts), dtype=np.float32)


# revision 8
# speedup vs baseline: 1.2369x; 1.2369x over previous
"""ColBERT pairwise-distill KL loss on 8 Trainium2 NeuronCores.

Strategy (doc-axis sharding): core k owns doc batches c in [8k, 8k+8).
Each core holds the full (transposed) query embeddings for student and
teacher in SBUF, streams its doc shard once from HBM, and computes the
full 64-row x 8-col block of both MaxSim score matrices:

    scores[b, c] = sum_n max_s  q[b, n, :] . d[c, s, :]

via float32r (fp22, full-rate) matmuls contracting over D=128 on the
partition axis.  The max over doc tokens runs on the vector engine with
a custom two-source fused max+max-reduce DVE op (2 PSUM elements per
lane-cycle, 2x the builtin tensor_reduce).  The sum over query tokens
is a ones-matrix matmul on the tensor engine.  Host-side work is only
sharding/layout of inputs and the [64,64]->scalar loss epilogue (exact
numpy float32 replica of the jax reference semantics, ~40K flops).
"""

import numpy as np

import concourse.bass as bass
import concourse.tile as tile
from concourse import bacc, mybir
from concourse import dve_ops
from concourse.bass_utils import run_bass_kernel_spmd
from concourse.dve_ops import DveOp
from concourse.dve_spec import C0, Spec, Src0, Src1, lower, maxx
from concourse.dve_uop import DveOpSpec

N_CORES = 8
B, N, S, D = 64, 32, 1024, 128
CPC = B // N_CORES          # doc batches per core
BPG = 4                     # query batches per matmul group (4*32 = 128 rows)
G = B // BPG                # 16 groups
NCOL = G * CPC              # 128 mx columns per side
ALPHA = 0.5
TEMPERATURE = 1.0
F32 = mybir.dt.float32
F32R = mybir.dt.float32r
BF16 = mybir.dt.bfloat16


def _ref_maxx2(in0, in1, c0, c1, c2):
    m = np.maximum(in0.astype(np.float32), in1.astype(np.float32))
    acc = np.maximum(m.reshape(m.shape[0], -1).max(axis=1, keepdims=True), c0)
    return m, acc


def _register_maxx2():
    """out = max(in0, in1); accum_out = max(s0, rowmax(out)).

    Registered through the supported custom-DVE extension point
    (dve_ops.OPS); the uop program ships inside the NEFF.
    """
    name = "MAXX2_REDUCE_ANT"
    if name in dve_ops._SUB_OPCODE_FOR_NAME:
        return next(o for o in dve_ops.OPS if o.name == name)
    spec = Spec(body=maxx(Src0, Src1), accum=maxx, accum_init=C0, reference=_ref_maxx2)
    row = dve_ops._CUSTOM_DVE_ROW_BASE + len(dve_ops.OPS)
    op = DveOp(name, spec, subdim=False, uops_sha={})
    dve_ops.OPS.append(op)
    dve_ops.CUSTOM_DVE_SPECS[name] = spec
    dve_ops._SUB_OPCODE_FOR_NAME[name] = row
    for ver in ("v3",):
        dve_ops._COMPILE_CACHE[(name, ver)] = DveOpSpec(
            name=name, opcode=row, uops=lower(spec, ver=ver), rd1_en=True
        )
    return op


def _emit(tc, nc, maxx2, qT_d, tqT_d, dT_d, tdT_d, ones4_d, out_d, ctx):
    sides = ((qT_d, dT_d), (tqT_d, tdT_d))

    resident = ctx.enter_context(tc.tile_pool(name="resident", bufs=1))
    scr = ctx.enter_context(tc.tile_pool(name="scr", bufs=2))

    ones_sb = resident.tile([D, BPG], F32, tag="ones", name="ones_sb")
    nc.sync.dma_start(ones_sb[:], ones4_d.ap())

    # Per-side resident operands + the per-(group, doc-batch) max matrix.
    qt_sb, dt_sb, mx_sb = [], [], []
    for si, (q_d, d_d) in enumerate(sides):
        qt = resident.tile([D, B * N], BF16, tag=f"qt{si}", name=f"qt{si}")
        nc.sync.dma_start(qt[:], q_d.ap())
        qt_sb.append(qt)
        dts = []
        for c in range(CPC):
            dt = resident.tile([D, S], BF16, tag=f"dt{si}_{c}", name=f"dt{si}_{c}")
            nc.sync.dma_start(dt[:], d_d.ap()[:, c, :])
            dts.append(dt)
        dt_sb.append(dts)
        mx_sb.append(resident.tile([D, NCOL], F32, tag=f"mx{si}", name=f"mx{si}"))

    half = S // 2
    main_psum = tc.tile_pool(name="psum", bufs=4, space="PSUM")
    psum = main_psum.__enter__()
    for si in range(2):
        for g in range(G):
            lhsT = qt_sb[si][:, g * 128 : (g + 1) * 128]
            for c in range(CPC):
                dt = dt_sb[si][c]
                ps = psum.tile([D, S], F32)
                nc.tensor.matmul(
                    ps[:, 0:half], lhsT, dt[:, 0:half],
                    start=True, stop=True,
                )
                nc.tensor.matmul(
                    ps[:, half:S], lhsT, dt[:, half:S],
                    start=True, stop=True,
                )
                col = g * CPC + c
                # DVE reads at most one PSUM operand per instruction; ScalarE
                # (otherwise idle) stages the second half into SBUF.
                sb_half = scr.tile([D, half], F32, name="sb_half")
                nc.scalar.copy(sb_half[:], ps[:, half:S])
                scratch = scr.tile([D, half], F32)
                nc.vector._custom_dve(
                    maxx2,
                    out=scratch[:],
                    in0=ps[:, 0:half],
                    in1=sb_half[:],
                    s0=-3.0e38,
                    accum_out=mx_sb[si][:, col : col + 1],
                )
    main_psum.__exit__(None, None, None)

    # Sum over the 32 query tokens inside each 32-partition block:
    # scores_part[p, col] for the 4 batches of each group.
    out_sb = resident.tile([BPG, 2 * NCOL], F32, tag="out", name="out_sb")
    with tc.tile_pool(name="psum_fin", bufs=1, space="PSUM") as psum_fin:
        psf = psum_fin.tile([BPG, 2 * NCOL], F32)
        for si in range(2):
            nc.tensor.matmul(
                psf[:, si * NCOL : (si + 1) * NCOL], ones_sb[:], mx_sb[si][:],
                start=True, stop=True,
            )
        nc.scalar.copy(out_sb[:], psf[:])
    nc.sync.dma_start(out_d.ap(), out_sb[:])


_PROGRAM = None


def _build_program():
    global _PROGRAM
    if _PROGRAM is not None:
        return _PROGRAM
    maxx2 = _register_maxx2()
    nc = bacc.Bacc("TRN2", target_bir_lowering=False, debug=False, num_devices=N_CORES)
    qT_d = nc.dram_tensor("qT", [D, B * N], BF16, kind="ExternalInput")
    tqT_d = nc.dram_tensor("tqT", [D, B * N], BF16, kind="ExternalInput")
    dT_d = nc.dram_tensor("dT", [D, CPC, S], BF16, kind="ExternalInput")
    tdT_d = nc.dram_tensor("tdT", [D, CPC, S], BF16, kind="ExternalInput")
    ones4_d = nc.dram_tensor("ones4", [D, BPG], F32, kind="ExternalInput")
    out_d = nc.dram_tensor("out", [BPG, 2 * NCOL], F32, kind="ExternalOutput")

    from contextlib import ExitStack

    with tile.TileContext(nc) as tc:
        with ExitStack() as ctx:
            _emit(tc, nc, maxx2, qT_d, tqT_d, dT_d, tdT_d, ones4_d, out_d, ctx)
    nc.compile()
    _PROGRAM = nc
    return nc


def _make_in_maps(query_embeddings, doc_embeddings, teacher_query_outputs,
                  teacher_doc_outputs):
    import ml_dtypes
    bf16 = ml_dtypes.bfloat16
    # [B, N, D] -> [D, B*N]; column index = 32*b + n = 128*g + 32*(b%4) + n.
    qT = np.ascontiguousarray(
        np.asarray(query_embeddings, dtype=np.float32).transpose(2, 0, 1).reshape(D, B * N)
    ).astype(bf16)
    tqT = np.ascontiguousarray(
        np.asarray(teacher_query_outputs, dtype=np.float32).transpose(2, 0, 1).reshape(D, B * N)
    ).astype(bf16)
    d_full = np.asarray(doc_embeddings, dtype=np.float32)
    td_full = np.asarray(teacher_doc_outputs, dtype=np.float32)
    ones4 = np.zeros((D, BPG), dtype=np.float32)
    for m in range(BPG):
        ones4[32 * m : 32 * (m + 1), m] = 1.0

    in_maps = []
    for k in range(N_CORES):
        sl = slice(k * CPC, (k + 1) * CPC)
        in_maps.append({
            "qT": qT,
            "tqT": tqT,
            "dT": np.ascontiguousarray(d_full[sl].transpose(2, 0, 1)).astype(bf16),
            "tdT": np.ascontiguousarray(td_full[sl].transpose(2, 0, 1)).astype(bf16),
            "ones4": ones4,
        })
    return in_maps


def _assemble_scores(results):
    """Per-core out [4, 256] -> full scores [64, 64] (student, teacher)."""
    s = np.empty((B, B), dtype=np.float32)
    ts = np.empty((B, B), dtype=np.float32)
    for k, r in enumerate(results):
        o = np.asarray(r["out"], dtype=np.float32)
        for si, dst in enumerate((s, ts)):
            blk = o[:, si * NCOL : (si + 1) * NCOL].reshape(BPG, G, CPC)
            # blk[p, g, cl] = scores[4g + p, 8k + cl]
            dst[:, k * CPC : (k + 1) * CPC] = (
                blk.transpose(1, 0, 2).reshape(B, CPC)
            )
    return s, ts


def _loss_from_scores(s, ts):
    """Exact numpy float32 replica of the jax reference epilogue."""
    pos = np.diagonal(s).copy()
    masked = s.copy()
    np.fill_diagonal(masked, -np.inf)
    neg = masked.max(axis=1)
    contrastive = np.logaddexp(np.float32(0.0), neg - pos).astype(np.float32).mean(
        dtype=np.float32
    )

    m = s.max(axis=1, keepdims=True)
    sh = s - m
    lse = np.log(np.exp(sh).sum(axis=1, keepdims=True, dtype=np.float32))
    slp = sh - lse

    mt = ts.max(axis=1, keepdims=True)
    e = np.exp(ts - mt)
    tp = (e / e.sum(axis=1, keepdims=True, dtype=np.float32)).astype(np.float32)
    with np.errstate(divide="ignore", invalid="ignore"):
        logtp = np.log(tp)
        kl = np.sum(tp * (logtp - slp), dtype=np.float32) / np.float32(B)
    kl = kl * np.float32(TEMPERATURE) ** 2
    return np.float32(contrastive + np.float32(ALPHA) * kl)


def run_scores(query_embeddings, doc_embeddings, teacher_query_outputs,
               teacher_doc_outputs, **spmd_kwargs):
    """Run the device program; returns (scores, teacher_scores, raw results)."""
    nc = _build_program()
    in_maps = _make_in_maps(
        query_embeddings, doc_embeddings, teacher_query_outputs, teacher_doc_outputs
    )
    res = run_bass_kernel_spmd(nc, in_maps, core_ids=list(range(N_CORES)), **spmd_kwargs)
    s, ts = _assemble_scores(res.results)
    return s, ts, res


def kernel(query_embeddings, doc_embeddings, teacher_query_outputs,
           teacher_doc_outputs):
    s, ts, _ = run_scores(
        query_embeddings, doc_embeddings, teacher_query_outputs, teacher_doc_outputs
    )
    return np.array(_loss_from_scores(s, ts), dtype=np.float32)


# revision 10
# speedup vs baseline: 1.4896x; 1.2043x over previous
"""ColBERT pairwise-distill KL loss on 8 Trainium2 NeuronCores.

Strategy (doc-axis sharding): core k owns doc batches c in [8k, 8k+8).
Each core holds the full (transposed) query embeddings for student and
teacher in SBUF, streams its doc shard once from HBM, and computes the
full 64-row x 8-col block of both MaxSim score matrices:

    scores[b, c] = sum_n max_s  q[b, n, :] . d[c, s, :]

via bf16 matmuls (fp32 PSUM accumulate) contracting over D=128 on the
partition axis.  The max over doc tokens runs on the vector engine with
a custom two-source fused max+max-reduce DVE op (2 sim elements per
lane-cycle, 2x the builtin tensor_reduce); the scalar engine stages one
PSUM half into SBUF per tile since the DVE can read only one PSUM
operand per instruction.  The sum over query tokens is a ones-matrix
matmul on the tensor engine.  Host-side work is only sharding/layout of
inputs and the [64,64]->scalar loss epilogue (exact numpy float32
replica of the jax reference semantics, ~40K flops; it reproduces the
reference's NaN, which arises from exact-zero teacher softmax
probabilities via 0 * log(0)).

Measured: ~214 us HW exec on 8 cores; scores match jax fp32 reference
to ~6e-4 relative (bf16 inputs); final loss NaN == reference NaN.
"""

import numpy as np

import concourse.bass as bass
import concourse.tile as tile
from concourse import bacc, mybir
from concourse import dve_ops
from concourse.bass_utils import run_bass_kernel_spmd
from concourse.dve_ops import DveOp
from concourse.dve_spec import C0, Spec, Src0, Src1, lower, maxx
from concourse.dve_uop import DveOpSpec

N_CORES = 8
B, N, S, D = 64, 32, 1024, 128
CPC = B // N_CORES          # doc batches per core
BPG = 4                     # query batches per matmul group (4*32 = 128 rows)
G = B // BPG                # 16 groups
NCOL = G * CPC              # 128 mx columns per side
ALPHA = 0.5
TEMPERATURE = 1.0
F32 = mybir.dt.float32
F32R = mybir.dt.float32r
BF16 = mybir.dt.bfloat16


def _ref_maxx2(in0, in1, c0, c1, c2):
    m = np.maximum(in0.astype(np.float32), in1.astype(np.float32))
    acc = np.maximum(m.reshape(m.shape[0], -1).max(axis=1, keepdims=True), c0)
    return m, acc


def _register_maxx2():
    """out = max(in0, in1); accum_out = max(s0, rowmax(out)).

    Registered through the supported custom-DVE extension point
    (dve_ops.OPS); the uop program ships inside the NEFF.
    """
    name = "MAXX2_REDUCE_ANT"
    if name in dve_ops._SUB_OPCODE_FOR_NAME:
        return next(o for o in dve_ops.OPS if o.name == name)
    spec = Spec(body=maxx(Src0, Src1), accum=maxx, accum_init=C0, reference=_ref_maxx2)
    row = dve_ops._CUSTOM_DVE_ROW_BASE + len(dve_ops.OPS)
    op = DveOp(name, spec, subdim=False, uops_sha={})
    dve_ops.OPS.append(op)
    dve_ops.CUSTOM_DVE_SPECS[name] = spec
    dve_ops._SUB_OPCODE_FOR_NAME[name] = row
    for ver in ("v3",):
        dve_ops._COMPILE_CACHE[(name, ver)] = DveOpSpec(
            name=name, opcode=row, uops=lower(spec, ver=ver), rd1_en=True
        )
    return op


def _emit(tc, nc, maxx2, qT_d, tqT_d, dT_d, tdT_d, ones4_d, out_d, ctx):
    sides = ((qT_d, dT_d), (tqT_d, tdT_d))

    resident = ctx.enter_context(tc.tile_pool(name="resident", bufs=1))
    scr = ctx.enter_context(tc.tile_pool(name="scr", bufs=4))

    ones_sb = resident.tile([D, BPG], F32, tag="ones", name="ones_sb")
    nc.sync.dma_start(ones_sb[:], ones4_d.ap())

    # Per-side resident operands + the per-(group, doc-batch) max matrix.
    qt_sb, dt_sb, mx_sb = [], [], []
    for si, (q_d, d_d) in enumerate(sides):
        qt = resident.tile([D, B * N], BF16, tag=f"qt{si}", name=f"qt{si}")
        nc.sync.dma_start(qt[:], q_d.ap())
        qt_sb.append(qt)
        dts = []
        for c in range(CPC):
            dt = resident.tile([D, S], BF16, tag=f"dt{si}_{c}", name=f"dt{si}_{c}")
            nc.sync.dma_start(dt[:], d_d.ap()[:, c, :])
            dts.append(dt)
        dt_sb.append(dts)
        mx_sb.append(resident.tile([D, NCOL], F32, tag=f"mx{si}", name=f"mx{si}"))

    half = S // 2
    main_psum = tc.tile_pool(name="psum", bufs=4, space="PSUM")
    psum = main_psum.__enter__()
    for si in range(2):
        for g in range(G):
            lhsT = qt_sb[si][:, g * 128 : (g + 1) * 128]
            for c in range(CPC):
                dt = dt_sb[si][c]
                ps = psum.tile([D, S], F32)
                nc.tensor.matmul(
                    ps[:, 0:half], lhsT, dt[:, 0:half],
                    start=True, stop=True,
                )
                nc.tensor.matmul(
                    ps[:, half:S], lhsT, dt[:, half:S],
                    start=True, stop=True,
                )
                col = g * CPC + c
                # DVE reads at most one PSUM operand per instruction; ScalarE
                # (otherwise idle) stages the second half into SBUF.
                sb_half = scr.tile([D, half], F32, name="sb_half")
                nc.scalar.copy(sb_half[:], ps[:, half:S])
                scratch = scr.tile([D, half], F32)
                nc.vector._custom_dve(
                    maxx2,
                    out=scratch[:],
                    in0=ps[:, 0:half],
                    in1=sb_half[:],
                    s0=-3.0e38,
                    accum_out=mx_sb[si][:, col : col + 1],
                )
    main_psum.__exit__(None, None, None)

    # Sum over the 32 query tokens inside each 32-partition block:
    # scores_part[p, col] for the 4 batches of each group.
    out_sb = resident.tile([BPG, 2 * NCOL], F32, tag="out", name="out_sb")
    with tc.tile_pool(name="psum_fin", bufs=1, space="PSUM") as psum_fin:
        psf = psum_fin.tile([BPG, 2 * NCOL], F32)
        for si in range(2):
            nc.tensor.matmul(
                psf[:, si * NCOL : (si + 1) * NCOL], ones_sb[:], mx_sb[si][:],
                start=True, stop=True,
            )
        nc.scalar.copy(out_sb[:], psf[:])
    nc.sync.dma_start(out_d.ap(), out_sb[:])


_PROGRAM = None


def _build_program():
    global _PROGRAM
    if _PROGRAM is not None:
        return _PROGRAM
    maxx2 = _register_maxx2()
    nc = bacc.Bacc("TRN2", target_bir_lowering=False, debug=False, num_devices=N_CORES)
    qT_d = nc.dram_tensor("qT", [D, B * N], BF16, kind="ExternalInput")
    tqT_d = nc.dram_tensor("tqT", [D, B * N], BF16, kind="ExternalInput")
    dT_d = nc.dram_tensor("dT", [D, CPC, S], BF16, kind="ExternalInput")
    tdT_d = nc.dram_tensor("tdT", [D, CPC, S], BF16, kind="ExternalInput")
    ones4_d = nc.dram_tensor("ones4", [D, BPG], F32, kind="ExternalInput")
    out_d = nc.dram_tensor("out", [BPG, 2 * NCOL], F32, kind="ExternalOutput")

    from contextlib import ExitStack

    with tile.TileContext(nc) as tc:
        with ExitStack() as ctx:
            _emit(tc, nc, maxx2, qT_d, tqT_d, dT_d, tdT_d, ones4_d, out_d, ctx)
    nc.compile()
    _PROGRAM = nc
    return nc


def _make_in_maps(query_embeddings, doc_embeddings, teacher_query_outputs,
                  teacher_doc_outputs):
    import ml_dtypes
    bf16 = ml_dtypes.bfloat16
    # [B, N, D] -> [D, B*N]; column index = 32*b + n = 128*g + 32*(b%4) + n.
    qT = np.ascontiguousarray(
        np.asarray(query_embeddings, dtype=np.float32).transpose(2, 0, 1).reshape(D, B * N)
    ).astype(bf16)
    tqT = np.ascontiguousarray(
        np.asarray(teacher_query_outputs, dtype=np.float32).transpose(2, 0, 1).reshape(D, B * N)
    ).astype(bf16)
    d_full = np.asarray(doc_embeddings, dtype=np.float32)
    td_full = np.asarray(teacher_doc_outputs, dtype=np.float32)
    ones4 = np.zeros((D, BPG), dtype=np.float32)
    for m in range(BPG):
        ones4[32 * m : 32 * (m + 1), m] = 1.0

    in_maps = []
    for k in range(N_CORES):
        sl = slice(k * CPC, (k + 1) * CPC)
        in_maps.append({
            "qT": qT,
            "tqT": tqT,
            "dT": np.ascontiguousarray(d_full[sl].transpose(2, 0, 1)).astype(bf16),
            "tdT": np.ascontiguousarray(td_full[sl].transpose(2, 0, 1)).astype(bf16),
            "ones4": ones4,
        })
    return in_maps


def _assemble_scores(results):
    """Per-core out [4, 256] -> full scores [64, 64] (student, teacher)."""
    s = np.empty((B, B), dtype=np.float32)
    ts = np.empty((B, B), dtype=np.float32)
    for k, r in enumerate(results):
        o = np.asarray(r["out"], dtype=np.float32)
        for si, dst in enumerate((s, ts)):
            blk = o[:, si * NCOL : (si + 1) * NCOL].reshape(BPG, G, CPC)
            # blk[p, g, cl] = scores[4g + p, 8k + cl]
            dst[:, k * CPC : (k + 1) * CPC] = (
                blk.transpose(1, 0, 2).reshape(B, CPC)
            )
    return s, ts


def _loss_from_scores(s, ts):
    """Exact numpy float32 replica of the jax reference epilogue."""
    pos = np.diagonal(s).copy()
    masked = s.copy()
    np.fill_diagonal(masked, -np.inf)
    neg = masked.max(axis=1)
    contrastive = np.logaddexp(np.float32(0.0), neg - pos).astype(np.float32).mean(
        dtype=np.float32
    )

    m = s.max(axis=1, keepdims=True)
    sh = s - m
    lse = np.log(np.exp(sh).sum(axis=1, keepdims=True, dtype=np.float32))
    slp = sh - lse

    mt = ts.max(axis=1, keepdims=True)
    e = np.exp(ts - mt)
    tp = (e / e.sum(axis=1, keepdims=True, dtype=np.float32)).astype(np.float32)
    with np.errstate(divide="ignore", invalid="ignore"):
        logtp = np.log(tp)
        kl = np.sum(tp * (logtp - slp), dtype=np.float32) / np.float32(B)
    kl = kl * np.float32(TEMPERATURE) ** 2
    return np.float32(contrastive + np.float32(ALPHA) * kl)


def run_scores(query_embeddings, doc_embeddings, teacher_query_outputs,
               teacher_doc_outputs, **spmd_kwargs):
    """Run the device program; returns (scores, teacher_scores, raw results)."""
    nc = _build_program()
    in_maps = _make_in_maps(
        query_embeddings, doc_embeddings, teacher_query_outputs, teacher_doc_outputs
    )
    res = run_bass_kernel_spmd(nc, in_maps, core_ids=list(range(N_CORES)), **spmd_kwargs)
    s, ts = _assemble_scores(res.results)
    return s, ts, res


def kernel(query_embeddings, doc_embeddings, teacher_query_outputs,
           teacher_doc_outputs):
    s, ts, _ = run_scores(
        query_embeddings, doc_embeddings, teacher_query_outputs, teacher_doc_outputs
    )
    return np.array(_loss_from_scores(s, ts), dtype=np.float32)
